# revision 35
# baseline (speedup 1.0000x reference)
"""GAU encoder block (ScaleNorm -> uv proj + silu -> squared-relu attention
-> output proj -> scaled residual) on 8 TRN2 NeuronCores.

Sharding: 4-way data parallel over batch x 2-way tensor parallel over the
expansion dim E. Core i handles (b = i // 2, h = i % 2): it computes the
full uv projection for its E-half (u/v halves sharded, the small s-dim
"base" head replicated), the full squared-relu attention matrix for batch b
(cheap, duplicated), attn@v for its E-half, and the output projection
against its E-half of o_w.  Each core returns a partial [D, T] output
(residual folded in on h==0 cores); the host sums the two partials per
batch and transposes back.

All activations are kept feature-major ([feature, token]) on-chip so every
matmul contraction lands on the partition axis with zero device-side
transposes; the host pre-transposes x / pos_enc / weights instead.  The
norm -> xn -> projections -> q/k -> scores chain is emitted chunk-pipelined
over 512-token column blocks, with the score tiles interleaved in a
triangle schedule, so the tensor engine runs one dense uninterrupted
matmul stream and all psum evictions hide under it.
"""

import numpy as np
import ml_dtypes

from contextlib import ExitStack

import concourse.bass as bass
import concourse.tile as tile
from concourse import mybir
from concourse.bass_utils import run_bass_kernel_spmd

# -- problem constants (hardcoded; kernel.py must be self-contained) --------
B = 4
T = 2048          # sequence length
D = 1024          # model dim
E = 2048          # expansion dim (2*D); each core handles EH = E//2
EH = E // 2
S = 128           # attention feature dim
EPS = 1e-5
INV_SQRT_S = 1.0 / float(np.sqrt(np.float32(S)))
INV_SQRT_D = float(D) ** -0.5

DT = T // 512     # 512-wide token chunks
ND = D // 128     # d-tiles
NE = EH // 128    # e-tiles of the local E-half
NT = T // 128     # t-tiles
TPC = NT // DT    # t-tiles per chunk
BF16 = mybir.dt.bfloat16
F32 = mybir.dt.float32

N_CORES = 8


def _split_excess_waits(nc, max_waits=1):
    """walrus's TPB_CTRL encoding rejects instructions carrying many sem
    waits (the Tile tail drain gets one per outstanding proc). Move excess
    waits onto InstNoOp carriers inserted just before, same engine."""
    for f in nc.m.functions:
        for bb in f.blocks:
            new_insts = []
            for ins in bb.instructions:
                si = ins.sync_info
                if si is not None and si.on_wait and len(si.on_wait) > max_waits:
                    excess = list(si.on_wait[max_waits:])
                    si.on_wait = list(si.on_wait[:max_waits])
                    for j, w in enumerate(excess):
                        new_insts.append(mybir.InstNoOp(
                            name=f"{ins.name}-waitsplit-{j}",
                            engine=ins.engine,
                            sync_info=mybir.SyncInfo(on_wait=[w], on_update=[]),
                        ))
                new_insts.append(ins)
            bb.instructions[:] = new_insts


def build_nc(reps=1):
    nc = bass.Bass()
    xTb = nc.declare_dram_parameter("xTb", [D, T], BF16, isOutput=False)
    wT = nc.declare_dram_parameter("wT", [D, EH + EH + S], BF16, isOutput=False)
    owT = nc.declare_dram_parameter("owT", [EH, D], BF16, isOutput=False)
    posq = nc.declare_dram_parameter("posq", [S, T], BF16, isOutput=False)
    posk = nc.declare_dram_parameter("posk", [S, T], BF16, isOutput=False)
    cst = nc.declare_dram_parameter("cst", [128, 16], F32, isOutput=False)
    outT = nc.declare_dram_parameter("outT", [D, T], F32, isOutput=True)

    Silu = mybir.ActivationFunctionType.Silu
    Sigmoid = mybir.ActivationFunctionType.Sigmoid
    Relu = mybir.ActivationFunctionType.Relu
    Sqrt = mybir.ActivationFunctionType.Sqrt
    Ident = mybir.ActivationFunctionType.Identity
    Alu = mybir.AluOpType

    with tile.TileContext(nc) as tc:
      for _rep in range(reps):
        with (
            tc.tile_pool(name="const", bufs=1) as const,
            tc.tile_pool(name="uT", bufs=1) as uT_pool,
            tc.tile_pool(name="vtm", bufs=1) as vtm_pool,
            tc.tile_pool(name="psum", bufs=5, space="PSUM") as psum,
            tc.tile_pool(name="A", bufs=1, side="right") as A_pool,
            tc.tile_pool(name="qk", bufs=1) as qk_pool,
            tc.tile_pool(name="srl", bufs=2) as srl_pool,
        ):
            cst_t = const.tile([128, 16], F32)
            nc.scalar.dma_start(out=cst_t[:], in_=cst[:])
            ones = const.tile([128, 1], BF16)
            nc.vector.memset(ones[:], 1.0)


            uT = uT_pool.tile([128, NE, T], BF16)       # u, feature-major
            vtm = vtm_pool.tile([128, NT, EH], BF16)    # v, token-major
            A = A_pool.tile([128, NT, T], BF16)         # attention matrix^T

            qbf = qk_pool.tile([S, T], BF16, name="qbf")
            kbf = qk_pool.tile([S, T], BF16, name="kbf")

            def scores_tile(kt, c2):
                ps = psum.tile([128, 512], F32, name="mm")
                nc.tensor.matmul(
                    ps[:], kbf[:, kt * 128:(kt + 1) * 128],
                    qbf[:, c2 * 512:(c2 + 1) * 512],
                    start=True, stop=True)
                sr = srl_pool.tile([128, 512], BF16)
                nc.scalar.activation(out=sr[:], in_=ps[:], func=Relu)
                nc.vector.tensor_mul(
                    out=A[:, kt, c2 * 512:(c2 + 1) * 512],
                    in0=sr[:], in1=sr[:])

            def silu_evict(dst, ps):
                nc.scalar.activation(out=dst, in_=ps[:], func=Silu)

            with ExitStack() as ph1:
                wT_pool = ph1.enter_context(tc.tile_pool(name="wT", bufs=1))
                xbc_pool = ph1.enter_context(tc.tile_pool(name="xbc", bufs=3))
                baseTc_pool = ph1.enter_context(tc.tile_pool(name="baseTc", bufs=1))
                frow_pool = ph1.enter_context(tc.tile_pool(name="frow", bufs=2))
                xsq_pool = ph1.enter_context(tc.tile_pool(name="xsq", bufs=2))
                posc_pool = ph1.enter_context(tc.tile_pool(name="posc", bufs=1))
                fsb_pool = ph1.enter_context(tc.tile_pool(name="fsb", bufs=1))
                ssq_psum = ph1.enter_context(tc.tile_pool(name="ssqp", bufs=2, space="PSUM"))
                fbc_psum = ph1.enter_context(tc.tile_pool(name="fbcp", bufs=1, space="PSUM"))
                w_t = wT_pool.tile([128, ND, EH + EH + S], BF16)
                xTb_r = xTb.rearrange("(dt p) t -> p dt t", p=128)
                # g / max(...) folds ln_g into the broadcast operand
                g_vec = const.tile([1, 128], F32)
                nc.vector.memset(g_vec[:], 1.0)
                nc.vector.tensor_scalar_mul(out=g_vec[:], in0=g_vec[:],
                                            scalar1=cst_t[0:1, 10:11])

                def norm_front(c):
                    """x chunk load + ssq matmuls + the f row chain.  Emitted
                    one chunk ahead so the scalar-chain latency hides under
                    the previous chunk's matmul stream."""
                    cs = slice(c * 512, (c + 1) * 512)
                    xbc = xbc_pool.tile([128, ND, 512], BF16)
                    if c == 0:
                        # split the first load so ssq starts after half
                        nc.sync.dma_start(out=xbc[:, 0:ND // 2, :],
                                          in_=xTb_r[:, 0:ND // 2, cs])
                        nc.sync.dma_start(out=xbc[:, ND // 2:, :],
                                          in_=xTb_r[:, ND // 2:, cs])
                    else:
                        nc.sync.dma_start(out=xbc[:], in_=xTb_r[:, :, cs])
                    if c == 0:
                        # weights land after the first x chunk: base head
                        # first (it is consumed first), then u, then v
                        wT_r = wT.rearrange("(dt p) f -> p dt f", p=128)
                        nc.sync.dma_start(out=w_t[:, :, 2 * EH:],
                                          in_=wT_r[:, :, 2 * EH:])
                        nc.sync.dma_start(out=w_t[:, :, 0:EH],
                                          in_=wT_r[:, :, 0:EH])
                        nc.sync.dma_start(out=w_t[:, :, EH:2 * EH],
                                          in_=wT_r[:, :, EH:2 * EH])
                    ssq = ssq_psum.tile([1, 512], F32, name="ssq")
                    for d in range(ND):
                        xsq = xsq_pool.tile([128, 512], BF16)
                        nc.vector.tensor_mul(out=xsq[:], in0=xbc[:, d, :],
                                             in1=xbc[:, d, :])
                        nc.tensor.matmul(ssq[:], ones[:], xsq[:],
                                         start=(d == 0), stop=(d == ND - 1))
                    # f = 1 / max(sqrt(ssq) * D^-0.5, eps)   (g in g_vec)
                    frow = frow_pool.tile([1, 512], F32, name="frow")
                    nc.scalar.activation(out=frow[:], in_=ssq[0:1, :],
                                         func=Sqrt)
                    nc.vector.tensor_scalar(
                        out=frow[:], in0=frow[:], scalar1=INV_SQRT_D,
                        scalar2=EPS, op0=Alu.mult, op1=Alu.max)
                    nc.vector.reciprocal(out=frow[:], in_=frow[:])
                    return xbc, frow

                carry = []
                nxt = norm_front(0)

                for c in range(DT):
                    cs = slice(c * 512, (c + 1) * 512)
                    # scores tiles that become computable after this chunk's
                    # q/k are written, drained incrementally next chunk
                    pend = ([(kt, c2) for kt in range(c * TPC, (c + 1) * TPC)
                             for c2 in range(c + 1)]
                            + [(kt, c) for kt in range(0, c * TPC)])
                    xbc, frow = nxt
                    # broadcast f to all 128 partitions: g_vec^T @ frow on
                    # the PE (fp32 single matmul, K=1), then xn in place
                    F_ps = fbc_psum.tile([128, 512], F32, name="fbc")
                    nc.tensor.matmul(F_ps[:], g_vec[:], frow[:],
                                     start=True, stop=True)
                    F_sb = fsb_pool.tile([128, 512], F32)
                    nc.scalar.activation(out=F_sb[:], in_=F_ps[:], func=Ident)
                    for d in range(ND):
                        nc.vector.tensor_mul(out=xbc[:, d, :], in0=xbc[:, d, :],
                                             in1=F_sb[:])

                    # -- projections for this chunk: base then u,
                    # with the previous chunk's scores interleaved --
                    drain_list = list(carry)
                    nsteps = NE + 1 + TPC * (EH // 512)
                    def drain_scores(step, q=[]):
                        if step == 0:
                            q[:] = drain_list
                        take = (len(drain_list) * (step + 1)) // nsteps - \
                               (len(drain_list) * step) // nsteps
                        for _ in range(take):
                            kt, c2 = q.pop(0)
                            scores_tile(kt, c2)
                    baseT_c = baseTc_pool.tile([128, 512], BF16)
                    for m in range(NE + 1):
                        drain_scores(m)
                        is_base = (m == 0)
                        w_cols = (slice(2 * EH, 2 * EH + S) if is_base
                                  else slice((m - 1) * 128, m * 128))
                        ps = psum.tile([128, 512], F32, name="mm")
                        for d in range(ND):
                            nc.tensor.matmul(
                                ps[:], w_t[:, d, w_cols], xbc[:, d, :],
                                start=(d == 0), stop=(d == ND - 1))
                        dst = (baseT_c[:] if is_base else uT[:, m - 1, cs])
                        silu_evict(dst, ps)

                    # prefetch next chunk's norm: its ssq matmuls slot in
                    # here and its f chain resolves during this chunk's v
                    if c + 1 < DT:
                        nxt = norm_front(c + 1)

                    # q/k for this chunk: per-head affine + positional
                    # encoding (gamma and the 1/sqrt(s) score scale fold
                    # into cst; beta+pos fold host-side into posq/posk)
                    posq_c = posc_pool.tile([S, 512], BF16, name="posq_c")
                    posk_c = posc_pool.tile([S, 512], BF16, name="posk_c")
                    nc.scalar.dma_start(out=posq_c[:], in_=posq[:, cs])
                    nc.scalar.dma_start(out=posk_c[:], in_=posk[:, cs])
                    nc.vector.scalar_tensor_tensor(
                        out=qbf[:, cs], in0=baseT_c[:], scalar=cst_t[:, 8:9],
                        in1=posq_c[:], op0=Alu.mult, op1=Alu.add)
                    nc.vector.scalar_tensor_tensor(
                        out=kbf[:, cs], in0=baseT_c[:], scalar=cst_t[:, 9:10],
                        in1=posk_c[:], op0=Alu.mult, op1=Alu.add)

                    # v (token-major) for this chunk's t-tiles
                    for tt in range(TPC):
                        t = c * TPC + tt
                        for ec in range(EH // 512):
                            drain_scores(NE + 1 + tt * (EH // 512) + ec)
                            ps = psum.tile([128, 512], F32, name="mm")
                            for d in range(ND):
                                nc.tensor.matmul(
                                    ps[:], xbc[:, d, tt * 128:(tt + 1) * 128],
                                    w_t[:, d, EH + ec * 512:EH + (ec + 1) * 512],
                                    start=(d == 0), stop=(d == ND - 1))
                            silu_evict(vtm[:, t, ec * 512:(ec + 1) * 512], ps)

                    carry = pend

            # ---- phase 3: attn@v, u*(.), output proj, residual ----
            with ExitStack() as ph3:
                mt_pool = ph3.enter_context(tc.tile_pool(name="mt", bufs=2, side="right"))
                owT_pool = ph3.enter_context(tc.tile_pool(name="owT", bufs=1, side="right"))
                xq_pool = ph3.enter_context(tc.tile_pool(name="xq", bufs=2))
                osb_pool = ph3.enter_context(tc.tile_pool(name="osb", bufs=3))
                psum3 = ph3.enter_context(tc.tile_pool(name="psum3", bufs=3, space="PSUM"))
                ow_t = owT_pool.tile([128, NE, D], BF16)
                nc.sync.dma_start(
                    out=ow_t[:], in_=owT.rearrange("(et p) f -> p et f", p=128))

                # the last chunk's deferred scores: (kt, c2==c) tiles must
                # land before av(c); the rest spread over c = 1..DT-1
                last = (DT - 1) * TPC
                p3_sc = {cc: [(kt, cc) for kt in range(last, NT)]
                         for cc in range(DT)}
                rest = [(kt, DT - 1) for kt in range(0, last)]
                nper = (len(rest) + DT - 2) // (DT - 1)
                for j in range(1, DT):
                    p3_sc[j] += rest[(j - 1) * nper:j * nper]

                def av_group(c, drain):
                    cs = slice(c * 512, (c + 1) * 512)
                    mt = mt_pool.tile([128, NE, 512], BF16)
                    q = list(drain)
                    for et in range(NE):
                        take = (len(drain) * (et + 1)) // NE - \
                               (len(drain) * et) // NE
                        for _ in range(take):
                            scores_tile(*q.pop(0))
                        ps = psum.tile([128, 512], F32, name="mm")
                        for kt in range(NT):
                            nc.tensor.matmul(
                                ps[:], vtm[:, kt, et * 128:(et + 1) * 128],
                                A[:, kt, cs],
                                start=(kt == 0), stop=(kt == NT - 1))
                        nc.vector.tensor_mul(
                            out=mt[:, et, :], in0=uT[:, et, cs], in1=ps[:])
                    return mt

                def out_group(c, mt):
                    cs = slice(c * 512, (c + 1) * 512)
                    xqt = xq_pool.tile([128, ND, 512], BF16)
                    nc.sync.dma_start(out=xqt[:], in_=xTb_r[:, :, cs])
                    for dt in range(ND):
                        ps = psum3.tile([128, 512], F32, name="mm3")
                        for et in range(NE):
                            nc.tensor.matmul(
                                ps[:], ow_t[:, et, dt * 128:(dt + 1) * 128],
                                mt[:, et, :],
                                start=(et == 0), stop=(et == NE - 1))
                        osb = osb_pool.tile([128, 512], F32)
                        nc.vector.scalar_tensor_tensor(
                            out=osb[:], in0=xqt[:, dt, :],
                            scalar=cst_t[:, dt:dt + 1],
                            in1=ps[:], op0=Alu.mult, op1=Alu.add)
                        nc.gpsimd.dma_start(
                            out=outT[dt * 128:(dt + 1) * 128, cs],
                            in_=osb[:])

                # software pipeline: out(c) issues behind av(c+1) so the
                # mt evictions of chunk c are long done when out(c) runs;
                # chunk c+1's deferred score tiles drain inside av(c)
                for kt, c2 in p3_sc[0]:
                    scores_tile(kt, c2)
                mts = [av_group(0, p3_sc[1])]
                for c in range(1, DT):
                    mts.append(av_group(c, p3_sc[c + 1] if c + 1 < DT else []))
                    out_group(c - 1, mts[c - 1])
                out_group(DT - 1, mts[DT - 1])

    return nc


def make_in_maps(x, pos_enc, uv_w, o_w, gamma, beta, ln_g, res_scale):
    bf16 = ml_dtypes.bfloat16
    c = np.float32(INV_SQRT_S)
    posT = np.ascontiguousarray(pos_enc.T)
    # q = base*(gamma0*c) + (pos + beta0)*c ; k = base*gamma1 + (pos + beta1)
    posq_a = ((posT + beta[0][:, None]) * c).astype(bf16)
    posk_a = (posT + beta[1][:, None]).astype(bf16)

    def make_cst(h):
        cstm = np.zeros((128, 16), dtype=np.float32)
        if h == 0:
            cstm[:, 0:ND] = res_scale.reshape(ND, 128).T
        cstm[:, 8] = gamma[0] * c
        cstm[:, 9] = gamma[1]
        cstm[:, 10] = np.float32(ln_g)
        return cstm
    cst0, cst1 = make_cst(0), make_cst(1)

    in_maps = []
    for i in range(N_CORES):
        b, h = divmod(i, 2)
        w_u = uv_w[h * EH:(h + 1) * EH]
        w_v = uv_w[E + h * EH:E + (h + 1) * EH]
        w_b = uv_w[2 * E:]
        wT_a = np.ascontiguousarray(
            np.concatenate([w_u, w_v, w_b], axis=0).T).astype(bf16)
        owT_a = np.ascontiguousarray(o_w[:, h * EH:(h + 1) * EH].T).astype(bf16)
        xTf = np.ascontiguousarray(x[b].T).astype(np.float32)
        in_maps.append({
            "xTb": xTf.astype(bf16),
            "wT": wT_a,
            "owT": owT_a,
            "posq": posq_a,
            "posk": posk_a,
            "cst": cst0 if h == 0 else cst1,
        })
    return in_maps


def combine(results):
    out = np.empty((B, T, D), dtype=np.float32)
    for b in range(B):
        out[b] = (results[2 * b]["outT"] + results[2 * b + 1]["outT"]).T
    return out


_NC_CACHE = {}


def _get_nc():
    if "nc" not in _NC_CACHE:
        nc = build_nc()
        _split_excess_waits(nc)   # only needed for the walrus compile path
        _NC_CACHE["nc"] = nc
    return _NC_CACHE["nc"]


def kernel(x, pos_enc, uv_w, o_w, gamma, beta, ln_g, res_scale):
    x = np.asarray(x, dtype=np.float32)
    in_maps = make_in_maps(
        x, np.asarray(pos_enc, np.float32), np.asarray(uv_w, np.float32),
        np.asarray(o_w, np.float32), np.asarray(gamma, np.float32),
        np.asarray(beta, np.float32), np.asarray(ln_g, np.float32),
        np.asarray(res_scale, np.float32))
    nc = _get_nc()
    res = run_bass_kernel_spmd(nc, in_maps, core_ids=list(range(N_CORES)))
    return combine(res.results)


# revision 36
# speedup vs baseline: 1.0017x; 1.0017x over previous
"""GAU encoder block (ScaleNorm -> uv proj + silu -> squared-relu attention
-> output proj -> scaled residual) on 8 TRN2 NeuronCores.

Sharding: 4-way data parallel over batch x 2-way tensor parallel over the
expansion dim E. Core i handles (b = i // 2, h = i % 2): it computes the
full uv projection for its E-half (u/v halves sharded, the small s-dim
"base" head replicated), the full squared-relu attention matrix for batch b
(cheap, duplicated), attn@v for its E-half, and the output projection
against its E-half of o_w.  Each core returns a partial [D, T] output
(residual folded in on h==0 cores); the host sums the two partials per
batch and transposes back.

All activations are kept feature-major ([feature, token]) on-chip so every
matmul contraction lands on the partition axis with zero device-side
transposes; the host pre-transposes x / pos_enc / weights instead.  The
norm -> xn -> projections -> q/k -> scores chain is emitted chunk-pipelined
over 512-token column blocks, with the score tiles interleaved in a
triangle schedule, so the tensor engine runs one dense uninterrupted
matmul stream and all psum evictions hide under it.
"""

import numpy as np
import ml_dtypes

from contextlib import ExitStack

import concourse.bass as bass
import concourse.tile as tile
from concourse import mybir
from concourse.bass_utils import run_bass_kernel_spmd

# -- problem constants (hardcoded; kernel.py must be self-contained) --------
B = 4
T = 2048          # sequence length
D = 1024          # model dim
E = 2048          # expansion dim (2*D); each core handles EH = E//2
EH = E // 2
S = 128           # attention feature dim
EPS = 1e-5
INV_SQRT_S = 1.0 / float(np.sqrt(np.float32(S)))
INV_SQRT_D = float(D) ** -0.5

DT = T // 512     # 512-wide token chunks
ND = D // 128     # d-tiles
NE = EH // 128    # e-tiles of the local E-half
NT = T // 128     # t-tiles
TPC = NT // DT    # t-tiles per chunk
BF16 = mybir.dt.bfloat16
F32 = mybir.dt.float32

N_CORES = 8


def _split_excess_waits(nc, max_waits=1):
    """walrus's TPB_CTRL encoding rejects instructions carrying many sem
    waits (the Tile tail drain gets one per outstanding proc). Move excess
    waits onto InstNoOp carriers inserted just before, same engine."""
    for f in nc.m.functions:
        for bb in f.blocks:
            new_insts = []
            for ins in bb.instructions:
                si = ins.sync_info
                if si is not None and si.on_wait and len(si.on_wait) > max_waits:
                    excess = list(si.on_wait[max_waits:])
                    si.on_wait = list(si.on_wait[:max_waits])
                    for j, w in enumerate(excess):
                        new_insts.append(mybir.InstNoOp(
                            name=f"{ins.name}-waitsplit-{j}",
                            engine=ins.engine,
                            sync_info=mybir.SyncInfo(on_wait=[w], on_update=[]),
                        ))
                new_insts.append(ins)
            bb.instructions[:] = new_insts


def build_nc(reps=1):
    nc = bass.Bass()
    xTb = nc.declare_dram_parameter("xTb", [D, T], BF16, isOutput=False)
    wT = nc.declare_dram_parameter("wT", [D, EH + EH + S], BF16, isOutput=False)
    owT = nc.declare_dram_parameter("owT", [EH, D], BF16, isOutput=False)
    posq = nc.declare_dram_parameter("posq", [S, T], BF16, isOutput=False)
    posk = nc.declare_dram_parameter("posk", [S, T], BF16, isOutput=False)
    cst = nc.declare_dram_parameter("cst", [128, 16], F32, isOutput=False)
    outT = nc.declare_dram_parameter("outT", [D, T], F32, isOutput=True)

    Silu = mybir.ActivationFunctionType.Silu
    Sigmoid = mybir.ActivationFunctionType.Sigmoid
    Relu = mybir.ActivationFunctionType.Relu
    Sqrt = mybir.ActivationFunctionType.Sqrt
    Ident = mybir.ActivationFunctionType.Identity
    Alu = mybir.AluOpType

    with tile.TileContext(nc) as tc:
      for _rep in range(reps):
        with (
            tc.tile_pool(name="const", bufs=1) as const,
            tc.tile_pool(name="uT", bufs=1) as uT_pool,
            tc.tile_pool(name="vtm", bufs=1) as vtm_pool,
            tc.tile_pool(name="psum", bufs=5, space="PSUM") as psum,
            tc.tile_pool(name="A", bufs=1, side="right") as A_pool,
            tc.tile_pool(name="qk", bufs=1) as qk_pool,
            tc.tile_pool(name="srl", bufs=2) as srl_pool,
        ):
            cst_t = const.tile([128, 16], F32)
            nc.scalar.dma_start(out=cst_t[:], in_=cst[:])
            ones = const.tile([128, 1], BF16)
            nc.vector.memset(ones[:], 1.0)


            uT = uT_pool.tile([128, NE, T], BF16)       # u, feature-major
            vtm = vtm_pool.tile([128, NT, EH], BF16)    # v, token-major
            A = A_pool.tile([128, NT, T], BF16)         # attention matrix^T

            qbf = qk_pool.tile([S, T], BF16, name="qbf")
            kbf = qk_pool.tile([S, T], BF16, name="kbf")

            def scores_tile(kt, c2):
                ps = psum.tile([128, 512], F32, name="mm")
                nc.tensor.matmul(
                    ps[:], kbf[:, kt * 128:(kt + 1) * 128],
                    qbf[:, c2 * 512:(c2 + 1) * 512],
                    start=True, stop=True)
                sr = srl_pool.tile([128, 512], BF16)
                nc.scalar.activation(out=sr[:], in_=ps[:], func=Relu)
                nc.vector.tensor_mul(
                    out=A[:, kt, c2 * 512:(c2 + 1) * 512],
                    in0=sr[:], in1=sr[:])

            def silu_evict(dst, ps):
                nc.scalar.activation(out=dst, in_=ps[:], func=Silu)

            with ExitStack() as ph1:
                wT_pool = ph1.enter_context(tc.tile_pool(name="wT", bufs=1))
                xbc_pool = ph1.enter_context(tc.tile_pool(name="xbc", bufs=3))
                baseTc_pool = ph1.enter_context(tc.tile_pool(name="baseTc", bufs=1))
                frow_pool = ph1.enter_context(tc.tile_pool(name="frow", bufs=2))
                xsq_pool = ph1.enter_context(tc.tile_pool(name="xsq", bufs=2))
                posc_pool = ph1.enter_context(tc.tile_pool(name="posc", bufs=1))
                fsb_pool = ph1.enter_context(tc.tile_pool(name="fsb", bufs=1))
                ssq_psum = ph1.enter_context(tc.tile_pool(name="ssqp", bufs=2, space="PSUM"))
                fbc_psum = ph1.enter_context(tc.tile_pool(name="fbcp", bufs=1, space="PSUM"))
                w_t = wT_pool.tile([128, ND, EH + EH + S], BF16)
                xTb_r = xTb.rearrange("(dt p) t -> p dt t", p=128)
                # g / max(...) folds ln_g into the broadcast operand
                g_vec = const.tile([1, 128], F32)
                nc.vector.memset(g_vec[:], 1.0)
                nc.vector.tensor_scalar_mul(out=g_vec[:], in0=g_vec[:],
                                            scalar1=cst_t[0:1, 10:11])

                def norm_front(c):
                    """x chunk load + ssq matmuls + the f row chain.  Emitted
                    one chunk ahead so the scalar-chain latency hides under
                    the previous chunk's matmul stream."""
                    cs = slice(c * 512, (c + 1) * 512)
                    xbc = xbc_pool.tile([128, ND, 512], BF16)
                    if c == 0:
                        # split the first load so ssq starts after half
                        nc.sync.dma_start(out=xbc[:, 0:ND // 2, :],
                                          in_=xTb_r[:, 0:ND // 2, cs])
                        nc.sync.dma_start(out=xbc[:, ND // 2:, :],
                                          in_=xTb_r[:, ND // 2:, cs])
                    else:
                        nc.sync.dma_start(out=xbc[:], in_=xTb_r[:, :, cs])
                    if c == 0:
                        # weights land after the first x chunk: base head
                        # first (it is consumed first), then u, then v
                        wT_r = wT.rearrange("(dt p) f -> p dt f", p=128)
                        nc.sync.dma_start(out=w_t[:, :, 2 * EH:],
                                          in_=wT_r[:, :, 2 * EH:])
                        nc.sync.dma_start(out=w_t[:, :, 0:EH],
                                          in_=wT_r[:, :, 0:EH])
                        nc.sync.dma_start(out=w_t[:, :, EH:2 * EH],
                                          in_=wT_r[:, :, EH:2 * EH])
                    ssq = ssq_psum.tile([1, 512], F32, name="ssq")
                    for d in range(ND):
                        xsq = xsq_pool.tile([128, 512], BF16)
                        nc.vector.tensor_mul(out=xsq[:], in0=xbc[:, d, :],
                                             in1=xbc[:, d, :])
                        nc.tensor.matmul(ssq[:], ones[:], xsq[:],
                                         start=(d == 0), stop=(d == ND - 1))
                    # f = 1 / max(sqrt(ssq) * D^-0.5, eps)   (g in g_vec)
                    frow = frow_pool.tile([1, 512], F32, name="frow")
                    nc.scalar.activation(out=frow[:], in_=ssq[0:1, :],
                                         func=Sqrt)
                    nc.vector.tensor_scalar(
                        out=frow[:], in0=frow[:], scalar1=INV_SQRT_D,
                        scalar2=EPS, op0=Alu.mult, op1=Alu.max)
                    nc.vector.reciprocal(out=frow[:], in_=frow[:])
                    return xbc, frow

                carry = []
                nxt = norm_front(0)

                for c in range(DT):
                    cs = slice(c * 512, (c + 1) * 512)
                    # scores tiles that become computable after this chunk's
                    # q/k are written, drained incrementally next chunk
                    pend = ([(kt, c2) for kt in range(c * TPC, (c + 1) * TPC)
                             for c2 in range(c + 1)]
                            + [(kt, c) for kt in range(0, c * TPC)])
                    xbc, frow = nxt
                    # broadcast f to all 128 partitions: g_vec^T @ frow on
                    # the PE (fp32 single matmul, K=1), then xn in place
                    F_ps = fbc_psum.tile([128, 512], F32, name="fbc")
                    nc.tensor.matmul(F_ps[:], g_vec[:], frow[:],
                                     start=True, stop=True)
                    F_sb = fsb_pool.tile([128, 512], F32)
                    nc.scalar.activation(out=F_sb[:], in_=F_ps[:], func=Ident)
                    for d in range(ND):
                        nc.vector.tensor_mul(out=xbc[:, d, :], in0=xbc[:, d, :],
                                             in1=F_sb[:])

                    # -- projections for this chunk: base then u,
                    # with the previous chunk's scores interleaved --
                    drain_list = list(carry)
                    nsteps = NE + 1 + TPC * (EH // 512)
                    def drain_scores(step, q=[]):
                        if step == 0:
                            q[:] = drain_list
                        take = (len(drain_list) * (step + 1)) // nsteps - \
                               (len(drain_list) * step) // nsteps
                        for _ in range(take):
                            kt, c2 = q.pop(0)
                            scores_tile(kt, c2)
                    baseT_c = baseTc_pool.tile([128, 512], BF16)
                    for m in range(NE + 1):
                        drain_scores(m)
                        is_base = (m == 0)
                        w_cols = (slice(2 * EH, 2 * EH + S) if is_base
                                  else slice((m - 1) * 128, m * 128))
                        ps = psum.tile([128, 512], F32, name="mm")
                        for d in range(ND):
                            nc.tensor.matmul(
                                ps[:], w_t[:, d, w_cols], xbc[:, d, :],
                                start=(d == 0), stop=(d == ND - 1))
                        dst = (baseT_c[:] if is_base else uT[:, m - 1, cs])
                        silu_evict(dst, ps)

                    # prefetch next chunk's norm: its ssq matmuls slot in
                    # here and its f chain resolves during this chunk's v
                    if c + 1 < DT:
                        nxt = norm_front(c + 1)

                    # q/k for this chunk: per-head affine + positional
                    # encoding (gamma and the 1/sqrt(s) score scale fold
                    # into cst; beta+pos fold host-side into posq/posk)
                    posq_c = posc_pool.tile([S, 512], BF16, name="posq_c")
                    posk_c = posc_pool.tile([S, 512], BF16, name="posk_c")
                    nc.scalar.dma_start(out=posq_c[:], in_=posq[:, cs])
                    nc.scalar.dma_start(out=posk_c[:], in_=posk[:, cs])
                    nc.vector.scalar_tensor_tensor(
                        out=qbf[:, cs], in0=baseT_c[:], scalar=cst_t[:, 8:9],
                        in1=posq_c[:], op0=Alu.mult, op1=Alu.add)
                    nc.vector.scalar_tensor_tensor(
                        out=kbf[:, cs], in0=baseT_c[:], scalar=cst_t[:, 9:10],
                        in1=posk_c[:], op0=Alu.mult, op1=Alu.add)

                    # v (token-major) for this chunk's t-tiles
                    for tt in range(TPC):
                        t = c * TPC + tt
                        for ec in range(EH // 512):
                            drain_scores(NE + 1 + tt * (EH // 512) + ec)
                            ps = psum.tile([128, 512], F32, name="mm")
                            for d in range(ND):
                                nc.tensor.matmul(
                                    ps[:], xbc[:, d, tt * 128:(tt + 1) * 128],
                                    w_t[:, d, EH + ec * 512:EH + (ec + 1) * 512],
                                    start=(d == 0), stop=(d == ND - 1))
                            silu_evict(vtm[:, t, ec * 512:(ec + 1) * 512], ps)

                    carry = pend

            # ---- phase 3: attn@v, u*(.), output proj, residual ----
            with ExitStack() as ph3:
                mt_pool = ph3.enter_context(tc.tile_pool(name="mt", bufs=2, side="right"))
                owT_pool = ph3.enter_context(tc.tile_pool(name="owT", bufs=1, side="right"))
                xq_pool = ph3.enter_context(tc.tile_pool(name="xq", bufs=2))
                osb_pool = ph3.enter_context(tc.tile_pool(name="osb", bufs=3))
                psum3 = ph3.enter_context(tc.tile_pool(name="psum3", bufs=3, space="PSUM"))
                ow_t = owT_pool.tile([128, NE, D], BF16)
                nc.sync.dma_start(
                    out=ow_t[:], in_=owT.rearrange("(et p) f -> p et f", p=128))

                # the last chunk's deferred scores: (kt, c2==c) tiles must
                # land before av(c); the rest spread over c = 1..DT-1
                last = (DT - 1) * TPC
                p3_sc = {cc: [(kt, cc) for kt in range(last, NT)]
                         for cc in range(DT)}
                rest = [(kt, DT - 1) for kt in range(0, last)]
                nper = (len(rest) + DT - 2) // (DT - 1)
                for j in range(1, DT):
                    p3_sc[j] += rest[(j - 1) * nper:j * nper]

                def av_group(c, drain):
                    cs = slice(c * 512, (c + 1) * 512)
                    mt = mt_pool.tile([128, NE, 512], BF16)
                    q = list(drain)
                    for et in range(NE):
                        take = (len(drain) * (et + 1)) // NE - \
                               (len(drain) * et) // NE
                        for _ in range(take):
                            scores_tile(*q.pop(0))
                        ps = psum.tile([128, 512], F32, name="mm")
                        for kt in range(NT):
                            nc.tensor.matmul(
                                ps[:], vtm[:, kt, et * 128:(et + 1) * 128],
                                A[:, kt, cs],
                                start=(kt == 0), stop=(kt == NT - 1))
                        nc.vector.tensor_mul(
                            out=mt[:, et, :], in0=uT[:, et, cs], in1=ps[:])
                    return mt

                def out_group(c, mt):
                    cs = slice(c * 512, (c + 1) * 512)
                    xqt = xq_pool.tile([128, ND, 512], BF16)
                    nc.sync.dma_start(out=xqt[:], in_=xTb_r[:, :, cs])
                    for dt in range(ND):
                        # the very last group is split in half so its
                        # eviction + store overlap the remaining matmuls
                        halves = ((0, 512),) if not (c == DT - 1 and
                                                     dt == ND - 1) \
                            else ((0, 256), (256, 256))
                        for h0, hw in halves:
                            ps = psum3.tile([128, 512], F32, name="mm3")
                            for et in range(NE):
                                nc.tensor.matmul(
                                    ps[:, :hw],
                                    ow_t[:, et, dt * 128:(dt + 1) * 128],
                                    mt[:, et, h0:h0 + hw],
                                    start=(et == 0), stop=(et == NE - 1))
                            osb = osb_pool.tile([128, 512], F32)
                            nc.vector.scalar_tensor_tensor(
                                out=osb[:, :hw], in0=xqt[:, dt, h0:h0 + hw],
                                scalar=cst_t[:, dt:dt + 1],
                                in1=ps[:, :hw], op0=Alu.mult, op1=Alu.add)
                            nc.gpsimd.dma_start(
                                out=outT[dt * 128:(dt + 1) * 128,
                                         c * 512 + h0:c * 512 + h0 + hw],
                                in_=osb[:, :hw])

                # software pipeline: out(c) issues behind av(c+1) so the
                # mt evictions of chunk c are long done when out(c) runs;
                # chunk c+1's deferred score tiles drain inside av(c)
                for kt, c2 in p3_sc[0]:
                    scores_tile(kt, c2)
                mts = [av_group(0, p3_sc[1])]
                for c in range(1, DT):
                    mts.append(av_group(c, p3_sc[c + 1] if c + 1 < DT else []))
                    out_group(c - 1, mts[c - 1])
                out_group(DT - 1, mts[DT - 1])

    return nc


def make_in_maps(x, pos_enc, uv_w, o_w, gamma, beta, ln_g, res_scale):
    bf16 = ml_dtypes.bfloat16
    c = np.float32(INV_SQRT_S)
    posT = np.ascontiguousarray(pos_enc.T)
    # q = base*(gamma0*c) + (pos + beta0)*c ; k = base*gamma1 + (pos + beta1)
    posq_a = ((posT + beta[0][:, None]) * c).astype(bf16)
    posk_a = (posT + beta[1][:, None]).astype(bf16)

    def make_cst(h):
        cstm = np.zeros((128, 16), dtype=np.float32)
        if h == 0:
            cstm[:, 0:ND] = res_scale.reshape(ND, 128).T
        cstm[:, 8] = gamma[0] * c
        cstm[:, 9] = gamma[1]
        cstm[:, 10] = np.float32(ln_g)
        return cstm
    cst0, cst1 = make_cst(0), make_cst(1)

    in_maps = []
    for i in range(N_CORES):
        b, h = divmod(i, 2)
        w_u = uv_w[h * EH:(h + 1) * EH]
        w_v = uv_w[E + h * EH:E + (h + 1) * EH]
        w_b = uv_w[2 * E:]
        wT_a = np.ascontiguousarray(
            np.concatenate([w_u, w_v, w_b], axis=0).T).astype(bf16)
        owT_a = np.ascontiguousarray(o_w[:, h * EH:(h + 1) * EH].T).astype(bf16)
        xTf = np.ascontiguousarray(x[b].T).astype(np.float32)
        in_maps.append({
            "xTb": xTf.astype(bf16),
            "wT": wT_a,
            "owT": owT_a,
            "posq": posq_a,
            "posk": posk_a,
            "cst": cst0 if h == 0 else cst1,
        })
    return in_maps


def combine(results):
    out = np.empty((B, T, D), dtype=np.float32)
    for b in range(B):
        out[b] = (results[2 * b]["outT"] + results[2 * b + 1]["outT"]).T
    return out


_NC_CACHE = {}


def _get_nc():
    if "nc" not in _NC_CACHE:
        nc = build_nc()
        _split_excess_waits(nc)   # only needed for the walrus compile path
        _NC_CACHE["nc"] = nc
    return _NC_CACHE["nc"]


def kernel(x, pos_enc, uv_w, o_w, gamma, beta, ln_g, res_scale):
    x = np.asarray(x, dtype=np.float32)
    in_maps = make_in_maps(
        x, np.asarray(pos_enc, np.float32), np.asarray(uv_w, np.float32),
        np.asarray(o_w, np.float32), np.asarray(gamma, np.float32),
        np.asarray(beta, np.float32), np.asarray(ln_g, np.float32),
        np.asarray(res_scale, np.float32))
    nc = _get_nc()
    res = run_bass_kernel_spmd(nc, in_maps, core_ids=list(range(N_CORES)))
    return combine(res.results)
